# revision 19
# baseline (speedup 1.0000x reference)
"""Trainium2 Bass kernel for nn_Cross_Attention_55671366091237.

Reference computation (B=4, N=2048, dim=512, H=8, dh=64):
    oq  = x @ W_fc + b_fc            # [B,N,64], modulates Q (bcast over heads)
    okv = y @ W_fc + b_fc            # [B,N,64], modulates K and V
    q,k,v = split(x @ W_qkv)         # each [B,N,512] -> heads [B,H,N,64]
    attn  = softmax(q*oq @ (k*okv)^T * dh^-0.5)
    out   = (attn @ (v*okv)) @ W_out + b_out

Sharding: 8 cores = 4 batches x 2 head-groups (4 heads each). Weights are
sliced per head-group host-side; x/y passed pre-transposed ([dim, N]).
Each core computes a partial output projection over its 4 heads; the host
sums the two partials per batch and adds b_out.

The kernel is ACT(exp)-bound: softmax needs 4*2048^2 exponentials per
core and the scalar engine is the only engine with exp, at 1 col/cycle
@1.2GHz. Everything is organized around a never-stalling stream of 128
[128,1024] exp instructions:
  - flat skewed pipeline over (pair, qt, kt): S^T(i) then exp(i) on ACT,
    attn@V(i-1) + S^T(i+1) hidden inside exp(i)'s window on the PE. The
    two S^T matmuls of a step auto-row-tile (partitions 0-63 / 64-127)
    and run concurrently in the PE array on HW.
  - the prefix is slice-granular: only slice 0 of the projections is
    emitted before the exp stream starts; the remaining okv/kmod/v4/oq/
    qmod slices, the pair-1 projections, softmax normalization, output
    projection and output DMA are "filler" items popped one per step,
    scheduled by Tile into PE/DVE slack under the exp stream.
  - PSUM: 3x[128,1024] shared S^T/filler slots (6 banks) + 2 accumulator
    banks = 8.
Numerics: the logit path (x, y, W_qkv, W_fc, modulations, Q/K) stays
f32r (full-rate fp32); only the post-softmax path (e, modulated V,
attention output, W_out) is bf16. Measured end-to-end ~4e-3 max-rel.
"""

import numpy as np

B, N, DIM = 4, 2048, 512
HEADS, DH = 8, 64
N_CORES = 8
SCALE = DH ** -0.5  # 0.125
NT = N // 128   # 16 key tiles of 128
NS = N // 512   # 4  slices of 512
DT = DIM // 128  # 4 contraction tiles

_RUNNER_CACHE = {}


# --------------------------------------------------------------------------
# Bass module
# --------------------------------------------------------------------------

def _build_nc(loop_n: int = 1):
    from collections import deque
    import contextlib

    import concourse.mybir as mybir
    from concourse import bacc
    from concourse.tile import TileContext
    from concourse.masks import make_identity

    fp32 = mybir.dt.float32
    f32r = mybir.dt.float32r
    bf16 = mybir.dt.bfloat16
    Exp = mybir.ActivationFunctionType.Exp

    nc = bacc.Bacc("TRN2", target_bir_lowering=False, debug=False)

    xT = nc.dram_tensor("xT", [DIM, N], fp32, kind="ExternalInput")
    yT = nc.dram_tensor("yT", [DIM, N], fp32, kind="ExternalInput")
    wq_d = nc.dram_tensor("wq", [DIM, 256], fp32, kind="ExternalInput")
    wk_d = nc.dram_tensor("wk", [DIM, 256], fp32, kind="ExternalInput")
    wv_d = nc.dram_tensor("wv", [DIM, 256], fp32, kind="ExternalInput")
    wfc_d = nc.dram_tensor("wfc", [DIM, DH], fp32, kind="ExternalInput")
    bfc_d = nc.dram_tensor("bfc", [DH, 1], fp32, kind="ExternalInput")
    wo_d = nc.dram_tensor("wo", [256, DIM], bf16, kind="ExternalInput")
    out_d = nc.dram_tensor("out", [N, DIM], fp32, kind="ExternalOutput")

    with TileContext(nc) as tc:
        with contextlib.ExitStack() as ctx:
            const = ctx.enter_context(tc.tile_pool(name="const", bufs=1))
            xtp = ctx.enter_context(tc.tile_pool(name="xtp", bufs=DT))
            ytp = ctx.enter_context(tc.tile_pool(name="ytp", bufs=DT))
            modp = ctx.enter_context(tc.tile_pool(name="modp", bufs=1))
            vp = ctx.enter_context(tc.tile_pool(name="vp", bufs=1))
            kp = ctx.enter_context(tc.tile_pool(name="kp", bufs=1))
            qmp = ctx.enter_context(tc.tile_pool(name="qmp", bufs=4))
            ep = ctx.enter_context(tc.tile_pool(name="ep", bufs=6))
            otp = ctx.enter_context(tc.tile_pool(name="otp", bufs=4))
            recp = ctx.enter_context(tc.tile_pool(name="recp", bufs=2))
            nsp = ctx.enter_context(tc.tile_pool(name="nsp", bufs=3))
            obp = ctx.enter_context(tc.tile_pool(name="obp", bufs=3))
            # PSUM: s 3x[128,1024]=6 banks + acc 2x[128,512]=2 banks
            sps = ctx.enter_context(
                tc.tile_pool(name="sps", bufs=3, space="PSUM"))
            accp = ctx.enter_context(
                tc.tile_pool(name="accp", bufs=2, space="PSUM"))

            def body(_i=None):
                # ---- weights / constants --------------------------------
                # DMA emission order tracks first-use order: wfc + slice-0
                # x/y chunks feed the inline prefix, wk/wq the first Q/K
                # projections; wv/wo and the remaining slices stream later.
                wfc2 = const.tile([128, DT, 128], f32r, tag="wfc2")
                wfc_r = wfc_d.bitcast(f32r).rearrange("(t p) f -> p t f", p=128)
                nc.sync.dma_start(wfc2[:, :, 0:DH], wfc_r)
                nc.sync.dma_start(wfc2[:, :, DH:128], wfc_r)
                bfc2 = const.tile([128, 1], fp32, tag="bfc2")
                nc.sync.dma_start(bfc2[0:DH, :], bfc_d[:, :])
                nc.sync.dma_start(bfc2[DH:128, :], bfc_d[:, :])

                xt, yt = [], []
                for t in range(DT):
                    xtile = xtp.tile([128, N], f32r, tag="xt",
                                     name=f"xt{t}")
                    xt.append(xtile)
                    ytile = ytp.tile([128, N], f32r, tag="yt",
                                     name=f"yt{t}")
                    yt.append(ytile)

                def xy_chunks(ns):
                    sl = slice(ns * 512, (ns + 1) * 512)
                    for t in range(DT):
                        tsl = slice(t * 128, (t + 1) * 128)
                        nc.sync.dma_start(yt[t][:, sl],
                                          yT.bitcast(f32r)[tsl, sl])
                        nc.sync.dma_start(xt[t][:, sl],
                                          xT.bitcast(f32r)[tsl, sl])

                xy_chunks(0)
                wk = const.tile([128, DT, 256], f32r, tag="wk")
                nc.sync.dma_start(wk[:, :, :],
                                  wk_d.bitcast(f32r).rearrange(
                                      "(t p) f -> p t f", p=128))
                wq = const.tile([128, DT, 256], f32r, tag="wq")
                nc.sync.dma_start(wq[:, :, :],
                                  wq_d.bitcast(f32r).rearrange(
                                      "(t p) f -> p t f", p=128))
                wv = const.tile([128, DT, 256], f32r, tag="wv")
                nc.sync.dma_start(wv[:, :, :],
                                  wv_d.bitcast(f32r).rearrange(
                                      "(t p) f -> p t f", p=128))
                for ns in (1, 2, 3):
                    xy_chunks(ns)
                wo = const.tile([128, 2, DIM], bf16, tag="wo")
                nc.sync.dma_start(wo[:, :, :],
                                  wo_d.rearrange("(t p) f -> p t f", p=128))
                ident = const.tile([128, 128], fp32, tag="ident")
                make_identity(nc, ident[:, :])
                ones1 = const.tile([128, 1], fp32, tag="ones1")
                nc.gpsimd.memset(ones1[:, :], 1.0)
                ones_row = const.tile([1, DH], bf16, tag="ones_row")
                nc.vector.tensor_copy(ones_row[:, :],
                                      ones1[0:1, :].broadcast_to((1, DH)))

                lp = nc.allow_low_precision

                okvT2 = modp.tile([128, N], fp32, tag="okvT2")
                oqT2 = modp.tile([128, N], fp32, tag="oqT2")
                okvn = modp.tile([128, NT, DH], bf16, tag="okvn")
                v4 = vp.tile([128, NT, 260], bf16, tag="v4")
                ones_b = ones1[:, :].unsqueeze(1).broadcast_to((128, NT, 1))
                v4h = v4[:, :, :].rearrange("p n (h c) -> p n h c", h=4)
                nc.vector.tensor_copy(v4h[:, :, :, DH:DH + 1],
                                      ones_b.unsqueeze(2).broadcast_to(
                                          (128, NT, 4, 1)))

                kmods = {}
                qmods = {}
                for p in (0, 1):
                    kmods[p] = kp.tile([128, N], f32r, tag=f"k{p}",
                                       name=f"km{p}")

                # ---- slice-granular producer items ----------------------
                # Each chunk is <= ~430ns of PE work so it fits the PE slack
                # inside one exp window; chunks use the m pool so they never
                # perturb the S^T psum slot rotation.
                mstate = {}
                sstate = {"tile": None, "half": 1, "n": 0}

                def s_region(nm):
                    # hand out [128,512] PSUM regions, two per s-pool tile
                    if sstate["half"] == 1:
                        sstate["n"] += 1
                        sstate["tile"] = sps.tile(
                            [128, 1024], fp32, tag="s",
                            name=f"sr{sstate['n']}_{nm}")
                        sstate["half"] = 0
                        return sstate["tile"][:, 0:512]
                    sstate["half"] = 1
                    return sstate["tile"][:, 512:1024]

                def proj_chunk(key, w, dest_cb, srcs, half):
                    # generic 4-matmul [128,512] projection split in halves
                    if half == 0:
                        ps = s_region(key[0])
                        mstate[key] = ps
                        ts = (0, 1)
                    else:
                        ps = mstate[key]
                        ts = (2, 3)
                    for t in ts:
                        nc.tensor.matmul(ps, w(t), srcs(t),
                                         start=(t == 0), stop=(t == DT - 1))
                    if half == 1:
                        dest_cb(ps)

                def okv_chunk(ns, half):
                    sl = slice(ns * 512, (ns + 1) * 512)

                    def dest(ps):
                        with lp(reason="f32r modulation"):
                            nc.vector.tensor_scalar_add(
                                okvT2[:, sl], ps, bfc2[:, :])
                    proj_chunk(("okv", ns), lambda t: wfc2[:, t, :], dest,
                               lambda t: yt[t][:, sl], half)

                def oq_chunk(ns, half):
                    sl = slice(ns * 512, (ns + 1) * 512)

                    def dest(ps):
                        with lp(reason="f32r modulation"):
                            nc.vector.tensor_scalar_add(
                                oqT2[:, sl], ps, bfc2[:, :])
                    proj_chunk(("oq", ns), lambda t: wfc2[:, t, :], dest,
                               lambda t: xt[t][:, sl], half)

                def k_chunk(p, ns, half):
                    km = kmods[p]
                    pf = slice(p * 128, (p + 1) * 128)
                    sl = slice(ns * 512, (ns + 1) * 512)

                    def dest(ps):
                        with lp(reason="f32r qk"):
                            nc.vector.tensor_mul(km[:, sl], ps,
                                                 okvT2[:, sl])
                    proj_chunk(("k", p, ns), lambda t: wk[:, t, pf], dest,
                               lambda t: xt[t][:, sl], half)

                def q_chunk(p, qt, half):
                    sl = slice(qt * 512, (qt + 1) * 512)
                    pf = slice(p * 128, (p + 1) * 128)
                    if half == 0:
                        qm = qmp.tile([128, 512], f32r, tag=f"q{p}",
                                      name=f"qm{p}_{qt}")
                        qmods[(p, qt)] = qm

                    def dest(ps):
                        with lp(reason="f32r qk"):
                            nc.vector.tensor_mul(qmods[(p, qt)][:, :],
                                                 ps, oqT2[:, sl])
                    proj_chunk(("q", p, qt), lambda t: wq[:, t, pf], dest,
                               lambda t: xt[t][:, sl], half)

                def tr_slice(ns):
                    # okv natural layout for 4 n-tiles via PE transpose
                    tps = s_region("tr")
                    for j in range(4):
                        nt = ns * 4 + j
                        nc.tensor.transpose(
                            tps[:, j * DH:(j + 1) * DH],
                            okvT2[0:DH, nt * 128:(nt + 1) * 128],
                            ident[0:DH, 0:DH])
                    with lp(reason="bf16 v-modulation"):
                        nc.vector.tensor_copy(
                            okvn[:, ns * 4:(ns + 1) * 4, :],
                            tps[:, 0:4 * DH].rearrange("p (n c) -> p n c",
                                                       n=4))

                def v4_chunk(ns, j):
                    # one n-tile (4 heads x 64) into half of a psum region;
                    # odd j modulates the completed pair
                    nt = ns * 4 + j
                    if j % 2 == 0:
                        psv = s_region("v4")
                        mstate[("v4", ns)] = psv
                    else:
                        psv = mstate[("v4", ns)]
                    csl = slice((j % 2) * 256, (j % 2) * 256 + 256)
                    for t in range(DT):
                        nc.tensor.matmul(psv[:, csl],
                                         xt[t][:, nt * 128:(nt + 1) * 128],
                                         wv[:, t, :],
                                         start=(t == 0), stop=(t == DT - 1))
                    if j % 2 == 1:
                        n0 = nt - 1
                        okb = okvn[:, n0:n0 + 2, :].unsqueeze(2).broadcast_to(
                            (128, 2, 4, DH))
                        with lp(reason="bf16 v-modulation"):
                            nc.vector.tensor_mul(
                                v4[:, n0:n0 + 2, :].rearrange(
                                    "p n (h c) -> p n h c",
                                    h=4)[:, :, :, 0:DH],
                                psv[:, :].rearrange(
                                    "p (n h c) -> p n h c", n=2, h=4),
                                okb)

                # ---- normalization / output projection items ------------
                ots = {}

                def norm_item(p, qt, h, acc):
                    # rec broadcast lands in the free partitions 64..127 of
                    # the accumulator's own PSUM bank — no new PSUM tile, so
                    # normalization cannot cycle through the s-pool rotation.
                    def run():
                        if (p, qt) not in ots:
                            ots[(p, qt)] = otp.tile(
                                [128, 512], bf16, tag=f"ot{p}",
                                name=f"ot{p}_{qt}")
                        ot = ots[(p, qt)]
                        rec = recp.tile([1, 512], bf16, tag="rec")
                        num = nsp.tile([DH, 512], bf16, tag="num")
                        with lp(reason="bf16 softmax normalization"):
                            nc.vector.tensor_copy(num[:, :], acc[0:DH, :])
                            nc.vector.reciprocal(rec[:, :], acc[64:65, :])
                            nc.tensor.matmul(acc[DH:128, :], ones_row[:, :],
                                             rec[:, :], start=True, stop=True)
                            nc.vector.tensor_mul(
                                ot[h * DH:(h + 1) * DH, :],
                                num[:, :], acc[DH:128, :])
                    return run

                def outproj_chunk(nt, half):
                    qt = nt // 4
                    j = nt % 4
                    jsl = slice(j * 128, (j + 1) * 128)
                    if half == 0:
                        ps = s_region("op")
                        mstate[("op", nt)] = ps
                        nc.tensor.matmul(ps, ots[(0, qt)][:, jsl],
                                         wo[:, 0, :], start=True, stop=False)
                    else:
                        ps = mstate[("op", nt)]
                        nsl = slice(nt * 128, (nt + 1) * 128)
                        nc.tensor.matmul(ps, ots[(1, qt)][:, jsl],
                                         wo[:, 1, :], start=False, stop=True)
                        ob = obp.tile([128, 512], fp32, tag="ob")
                        nc.vector.tensor_copy(ob[:, :], ps)
                        nc.sync.dma_start(out_d[nsl, :], ob[:, :])

                # ---- inline prefix: ALL projections -----------------------
                # The exp stream is the wall-clock backbone; anything with a
                # PE+DVE chain that could stall it runs before step 0. Only
                # the output projection (which depends on stream results)
                # interleaves, as 1-matmul chunks.
                okv_chunk(0, 0); okv_chunk(0, 1)
                oq_chunk(0, 0); oq_chunk(0, 1)
                k_chunk(0, 0, 0); k_chunk(0, 0, 1)
                q_chunk(0, 0, 0); q_chunk(0, 0, 1)
                tr_slice(0)
                for j in range(4):
                    v4_chunk(0, j)
                for ns in (1, 2, 3):
                    okv_chunk(ns, 0); okv_chunk(ns, 1)
                    k_chunk(0, ns, 0); k_chunk(0, ns, 1)
                    tr_slice(ns)
                    for j in range(4):
                        v4_chunk(ns, j)
                for qt in (1, 2, 3):
                    oq_chunk(qt, 0); oq_chunk(qt, 1)
                    q_chunk(0, qt, 0); q_chunk(0, qt, 1)
                for ns in range(NS):
                    k_chunk(1, ns, 0); k_chunk(1, ns, 1)
                for qt in range(4):
                    q_chunk(1, qt, 0); q_chunk(1, qt, 1)

                fill = deque()
                urgent = deque()
                sstate["half"] = 1  # stream-phase regions start fresh tiles

                # ---- attention: flat skewed pipeline --------------------
                steps = [(p, qt, kt)
                         for p in (0, 1) for qt in range(4)
                         for kt in range(NT)]
                prev = None
                accs = None
                for idx, (p, qt, kt) in enumerate(steps):
                    km = kmods[p]
                    qm = qmods[(p, qt)]
                    ksl = slice(kt * 128, (kt + 1) * 128)
                    sp = sps.tile([128, 1024], fp32, tag="s",
                                  name=f"sp{idx}")
                    nc.tensor.matmul(sp[:, 0:512], km[0:DH, ksl],
                                     qm[0:DH, :], start=True, stop=True)
                    nc.tensor.matmul(sp[:, 512:1024], km[DH:128, ksl],
                                     qm[DH:128, :], start=True, stop=True)
                    if prev is not None:
                        pp, pqt, pkt, pe, pa0, pa1 = prev
                        nc.tensor.matmul(pa0[0:65, :],
                                         v4[:, pkt, pp * 130:pp * 130 + 65],
                                         pe[:, 0:512],
                                         start=(pkt == 0), stop=(pkt == NT - 1))
                        nc.tensor.matmul(pa1[0:65, :],
                                         v4[:, pkt,
                                            pp * 130 + 65:pp * 130 + 130],
                                         pe[:, 512:1024],
                                         start=(pkt == 0), stop=(pkt == NT - 1))
                        if pkt == NT - 1:
                            urgent.append(norm_item(pp, pqt, 0, pa0))
                            urgent.append(norm_item(pp, pqt, 1, pa1))
                            if pp == 1:
                                for nt in range(pqt * 4, pqt * 4 + 4):
                                    fill.append(
                                        lambda nt=nt: outproj_chunk(nt, 0))
                                    fill.append(
                                        lambda nt=nt: outproj_chunk(nt, 1))
                    if kt == 0:
                        accs = (accp.tile([128, 512], fp32, tag="acc",
                                          name=f"acc0_{p}_{qt}"),
                                accp.tile([128, 512], fp32, tag="acc",
                                          name=f"acc1_{p}_{qt}"))
                    e = ep.tile([128, 1024], bf16, tag="e")
                    nc.scalar.activation(e[:, :], sp[:, :], Exp,
                                         scale=float(SCALE))
                    prev = (p, qt, kt, e, accs[0], accs[1])
                    if urgent:
                        urgent.popleft()()
                    elif fill:
                        fill.popleft()()

                # tail: final attn@V + remaining fillers
                pp, pqt, pkt, pe, pa0, pa1 = prev
                nc.tensor.matmul(pa0[0:65, :],
                                 v4[:, pkt, pp * 130:pp * 130 + 65],
                                 pe[:, 0:512], start=False, stop=True)
                nc.tensor.matmul(pa1[0:65, :],
                                 v4[:, pkt, pp * 130 + 65:pp * 130 + 130],
                                 pe[:, 512:1024], start=False, stop=True)
                urgent.append(norm_item(pp, pqt, 0, pa0))
                urgent.append(norm_item(pp, pqt, 1, pa1))
                for nt in range(pqt * 4, pqt * 4 + 4):
                    fill.append(lambda nt=nt: outproj_chunk(nt, 0))
                    fill.append(lambda nt=nt: outproj_chunk(nt, 1))
                while urgent:
                    urgent.popleft()()
                while fill:
                    fill.popleft()()

            if loop_n > 1:
                with tc.For_i(0, loop_n, 1) as _i:
                    body(_i)
            else:
                body()

    nc.compile()
    return nc


# --------------------------------------------------------------------------
# PJRT SPMD runner (axon path) — keeps the jitted callable for reuse
# --------------------------------------------------------------------------

class _SpmdRunner:
    def __init__(self, nc, n_cores):
        import jax
        from jax.sharding import Mesh, PartitionSpec, NamedSharding
        from jax.experimental.shard_map import shard_map
        import concourse.mybir as mybir
        from concourse import bass2jax
        from concourse.bass2jax import _bass_exec_p, install_neuronx_cc_hook

        install_neuronx_cc_hook()
        self.jax = jax
        self.nc = nc
        self.n_cores = n_cores
        pname = nc.partition_id_tensor.name if nc.partition_id_tensor else None
        in_names, out_names, out_avals, zero_shapes = [], [], [], []
        for alloc in nc.m.functions[0].allocations:
            if not isinstance(alloc, mybir.MemoryLocationSet):
                continue
            name = alloc.memorylocations[0].name
            if alloc.kind == "ExternalInput":
                if name != pname:
                    in_names.append(name)
            elif alloc.kind == "ExternalOutput":
                out_names.append(name)
                shape = tuple(alloc.tensor_shape)
                dtype = mybir.dt.np(alloc.dtype)
                out_avals.append(jax.core.ShapedArray(shape, dtype))
                zero_shapes.append((shape, dtype))
        self.n_params = len(in_names)
        self.in_names = list(in_names)
        self.out_names = out_names
        self.out_avals = out_avals
        all_names = in_names + out_names
        if pname is not None:
            all_names.append(pname)

        def _body(*args):
            operands = list(args)
            if pname is not None:
                operands.append(bass2jax.partition_id_tensor())
            return tuple(_bass_exec_p.bind(
                *operands, out_avals=tuple(out_avals),
                in_names=tuple(all_names), out_names=tuple(out_names),
                lowering_input_output_aliases=(),
                sim_require_finite=True, sim_require_nnan=True, nc=nc))

        devices = jax.devices()[:n_cores]
        self.mesh = Mesh(np.asarray(devices), ("core",))
        n_outs = len(out_avals)
        in_specs = (PartitionSpec("core"),) * (self.n_params + n_outs)
        out_specs = (PartitionSpec("core"),) * n_outs
        donate = tuple(range(self.n_params, self.n_params + n_outs))
        self.sharding = NamedSharding(self.mesh, PartitionSpec("core"))
        self.sharded = jax.jit(
            shard_map(_body, mesh=self.mesh, in_specs=in_specs,
                      out_specs=out_specs, check_rep=False),
            donate_argnums=donate, keep_unused=True)
        zs = [(n_cores * s[0], *s[1:]) for s, _ in zero_shapes]
        zd = [d for _, d in zero_shapes]
        self._mkzeros = jax.jit(
            lambda: tuple(jax.numpy.zeros(s, d) for s, d in zip(zs, zd)),
            out_shardings=tuple(self.sharding for _ in zs))

    def put_inputs(self, in_maps):
        concat = [np.concatenate(
            [np.ascontiguousarray(in_maps[c][n]) for c in range(self.n_cores)],
            axis=0) for n in self.in_names]
        return [self.jax.device_put(a, self.sharding) for a in concat]

    def run(self, in_dev):
        outs = self.sharded(*in_dev, *self._mkzeros())
        self.jax.block_until_ready(outs)
        return outs

    def results(self, outs):
        res = []
        for c in range(self.n_cores):
            d = {}
            for i, name in enumerate(self.out_names):
                full = np.asarray(outs[i])
                d[name] = full.reshape(self.n_cores,
                                       *self.out_avals[i].shape)[c]
            res.append(d)
        return res


def _get_runner(loop_n: int = 1):
    if loop_n not in _RUNNER_CACHE:
        nc = _build_nc(loop_n)
        _RUNNER_CACHE[loop_n] = _SpmdRunner(nc, N_CORES)
    return _RUNNER_CACHE[loop_n]


# --------------------------------------------------------------------------
# host-side shard / gather
# --------------------------------------------------------------------------

def _shard_inputs(x, y, W_qkv, W_fc, b_fc, W_out):
    import ml_dtypes
    bf = ml_dtypes.bfloat16
    in_maps = []
    x = np.asarray(x)
    y = np.asarray(y)
    W_qkv = np.asarray(W_qkv)
    W_fc = np.ascontiguousarray(np.asarray(W_fc, dtype=np.float32))
    b_fc = np.asarray(b_fc, dtype=np.float32).reshape(DH, 1)
    W_out = np.asarray(W_out)
    xTb = [np.ascontiguousarray(x[b].T) for b in range(B)]
    yTb = [np.ascontiguousarray(y[b].T) for b in range(B)]
    for c in range(N_CORES):
        b, g = c // 2, c % 2
        hs = slice(g * 256, (g + 1) * 256)
        in_maps.append({
            "xT": xTb[b],
            "yT": yTb[b],
            "wq": np.ascontiguousarray(W_qkv[:, hs]),
            "wk": np.ascontiguousarray(W_qkv[:, 512:][:, hs]),
            "wv": np.ascontiguousarray(W_qkv[:, 1024:][:, hs]),
            "wfc": W_fc,
            "bfc": b_fc,
            "wo": np.ascontiguousarray(W_out[hs, :]).astype(bf),
        })
    return in_maps


def kernel(x, y, W_qkv, W_fc, b_fc, W_out, b_out):
    runner = _get_runner(1)
    in_maps = _shard_inputs(x, y, W_qkv, W_fc, b_fc, W_out)
    in_dev = runner.put_inputs(in_maps)
    res = runner.results(runner.run(in_dev))
    b_out = np.asarray(b_out, dtype=np.float32)
    out = np.empty((B, N, DIM), dtype=np.float32)
    for b in range(B):
        out[b] = res[2 * b]["out"] + res[2 * b + 1]["out"] + b_out
    return out


# revision 21
# speedup vs baseline: 1.1258x; 1.1258x over previous
"""Trainium2 Bass kernel for nn_Cross_Attention_55671366091237.

Reference computation (B=4, N=2048, dim=512, H=8, dh=64):
    oq  = x @ W_fc + b_fc            # [B,N,64], modulates Q (bcast over heads)
    okv = y @ W_fc + b_fc            # [B,N,64], modulates K and V
    q,k,v = split(x @ W_qkv)         # each [B,N,512] -> heads [B,H,N,64]
    attn  = softmax(q*oq @ (k*okv)^T * dh^-0.5)
    out   = (attn @ (v*okv)) @ W_out + b_out

Sharding: 8 cores = 4 batches x 2 head-groups (4 heads each). Weights are
sliced per head-group host-side; x/y passed pre-transposed ([dim, N]).
Each core computes a partial output projection over its 4 heads; the host
sums the two partials per batch and adds b_out.

The kernel is ACT(exp)-bound: softmax needs 4*2048^2 exponentials per
core and the scalar engine is the only engine with exp, at 1 col/cycle
@1.2GHz. Everything is organized around a never-stalling stream of 128
[128,1024] exp instructions:
  - flat skewed pipeline over (pair, qt, kt): S^T(i) then exp(i) on ACT,
    attn@V(i-1) + S^T(i+1) hidden inside exp(i)'s window on the PE. The
    two S^T matmuls of a step auto-row-tile (partitions 0-63 / 64-127)
    and run concurrently in the PE array on HW.
  - the prefix is slice-granular: only slice 0 of the projections is
    emitted before the exp stream starts; the remaining okv/kmod/v4/oq/
    qmod slices, the pair-1 projections, softmax normalization, output
    projection and output DMA are "filler" items popped one per step,
    scheduled by Tile into PE/DVE slack under the exp stream.
  - PSUM: 3x[128,1024] shared S^T/filler slots (6 banks) + 2 accumulator
    banks = 8.
Numerics: the logit path (x, y, W_qkv, W_fc, modulations, Q/K) stays
f32r (full-rate fp32); only the post-softmax path (e, modulated V,
attention output, W_out) is bf16. Measured end-to-end ~4e-3 max-rel.
"""

import numpy as np

B, N, DIM = 4, 2048, 512
HEADS, DH = 8, 64
N_CORES = 8
SCALE = DH ** -0.5  # 0.125
NT = N // 128   # 16 key tiles of 128
NS = N // 512   # 4  slices of 512
DT = DIM // 128  # 4 contraction tiles

_RUNNER_CACHE = {}


# --------------------------------------------------------------------------
# Bass module
# --------------------------------------------------------------------------

def _build_nc(loop_n: int = 1):
    from collections import deque
    import contextlib

    import concourse.mybir as mybir
    from concourse import bacc
    from concourse.tile import TileContext
    from concourse.masks import make_identity

    fp32 = mybir.dt.float32
    f32r = mybir.dt.float32r
    bf16 = mybir.dt.bfloat16
    Exp = mybir.ActivationFunctionType.Exp

    nc = bacc.Bacc("TRN2", target_bir_lowering=False, debug=False)

    xT = nc.dram_tensor("xT", [DIM, N], fp32, kind="ExternalInput")
    yT = nc.dram_tensor("yT", [DIM, N], fp32, kind="ExternalInput")
    wq_d = nc.dram_tensor("wq", [DIM, 256], fp32, kind="ExternalInput")
    wk_d = nc.dram_tensor("wk", [DIM, 256], fp32, kind="ExternalInput")
    wv_d = nc.dram_tensor("wv", [DIM, 256], fp32, kind="ExternalInput")
    wfc_d = nc.dram_tensor("wfc", [DIM, DH], fp32, kind="ExternalInput")
    bfc_d = nc.dram_tensor("bfc", [DH, 1], fp32, kind="ExternalInput")
    wo_d = nc.dram_tensor("wo", [256, DIM], bf16, kind="ExternalInput")
    out_d = nc.dram_tensor("out", [N, DIM], fp32, kind="ExternalOutput")

    with TileContext(nc) as tc:
        with contextlib.ExitStack() as ctx:
            const = ctx.enter_context(tc.tile_pool(name="const", bufs=1))
            xtp = ctx.enter_context(tc.tile_pool(name="xtp", bufs=DT))
            ytp = ctx.enter_context(tc.tile_pool(name="ytp", bufs=DT))
            modp = ctx.enter_context(tc.tile_pool(name="modp", bufs=1))
            vp = ctx.enter_context(tc.tile_pool(name="vp", bufs=1))
            kp = ctx.enter_context(tc.tile_pool(name="kp", bufs=1))
            qmp = ctx.enter_context(tc.tile_pool(name="qmp", bufs=4))
            ep = ctx.enter_context(tc.tile_pool(name="ep", bufs=6))
            otp = ctx.enter_context(tc.tile_pool(name="otp", bufs=4))
            recp = ctx.enter_context(tc.tile_pool(name="recp", bufs=2))
            nsp = ctx.enter_context(tc.tile_pool(name="nsp", bufs=3))
            obp = ctx.enter_context(tc.tile_pool(name="obp", bufs=3))
            # PSUM: s 2x[128,1024]=4 banks + m 2x[128,512]=2 + acc 2 = 8
            sps = ctx.enter_context(
                tc.tile_pool(name="sps", bufs=2, space="PSUM"))
            mixp = ctx.enter_context(
                tc.tile_pool(name="mixp", bufs=2, space="PSUM"))
            accp = ctx.enter_context(
                tc.tile_pool(name="accp", bufs=2, space="PSUM"))

            def body(_i=None):
                # ---- weights / constants --------------------------------
                # DMA emission order tracks first-use order: wfc + slice-0
                # x/y chunks feed the inline prefix, wk/wq the first Q/K
                # projections; wv/wo and the remaining slices stream later.
                wfc2 = const.tile([128, DT, 128], f32r, tag="wfc2")
                wfc_r = wfc_d.bitcast(f32r).rearrange("(t p) f -> p t f", p=128)
                nc.sync.dma_start(wfc2[:, :, 0:DH], wfc_r)
                nc.sync.dma_start(wfc2[:, :, DH:128], wfc_r)
                bfc2 = const.tile([128, 1], fp32, tag="bfc2")
                nc.sync.dma_start(bfc2[0:DH, :], bfc_d[:, :])
                nc.sync.dma_start(bfc2[DH:128, :], bfc_d[:, :])

                xt, yt = [], []
                for t in range(DT):
                    xtile = xtp.tile([128, N], f32r, tag="xt",
                                     name=f"xt{t}")
                    xt.append(xtile)
                    ytile = ytp.tile([128, N], f32r, tag="yt",
                                     name=f"yt{t}")
                    yt.append(ytile)

                def xy_chunks(ns):
                    sl = slice(ns * 512, (ns + 1) * 512)
                    for t in range(DT):
                        tsl = slice(t * 128, (t + 1) * 128)
                        nc.sync.dma_start(yt[t][:, sl],
                                          yT.bitcast(f32r)[tsl, sl])
                        nc.sync.dma_start(xt[t][:, sl],
                                          xT.bitcast(f32r)[tsl, sl])

                xy_chunks(0)
                wk = const.tile([128, DT, 256], f32r, tag="wk")
                nc.sync.dma_start(wk[:, :, :],
                                  wk_d.bitcast(f32r).rearrange(
                                      "(t p) f -> p t f", p=128))
                wq = const.tile([128, DT, 256], f32r, tag="wq")
                nc.sync.dma_start(wq[:, :, :],
                                  wq_d.bitcast(f32r).rearrange(
                                      "(t p) f -> p t f", p=128))
                wv = const.tile([128, DT, 256], f32r, tag="wv")
                nc.sync.dma_start(wv[:, :, :],
                                  wv_d.bitcast(f32r).rearrange(
                                      "(t p) f -> p t f", p=128))
                for ns in (1, 2, 3):
                    xy_chunks(ns)
                wo = const.tile([128, 2, DIM], bf16, tag="wo")
                nc.sync.dma_start(wo[:, :, :],
                                  wo_d.rearrange("(t p) f -> p t f", p=128))
                ident = const.tile([128, 128], fp32, tag="ident")
                make_identity(nc, ident[:, :])
                ones1 = const.tile([128, 1], fp32, tag="ones1")
                nc.gpsimd.memset(ones1[:, :], 1.0)
                ones_row = const.tile([1, DH], bf16, tag="ones_row")
                nc.vector.tensor_copy(ones_row[:, :],
                                      ones1[0:1, :].broadcast_to((1, DH)))

                lp = nc.allow_low_precision

                okvT2 = modp.tile([128, N], fp32, tag="okvT2")
                oqT2 = modp.tile([128, N], fp32, tag="oqT2")
                okvn = modp.tile([128, NT, DH], bf16, tag="okvn")
                v4 = vp.tile([128, NT, 260], bf16, tag="v4")
                ones_b = ones1[:, :].unsqueeze(1).broadcast_to((128, NT, 1))
                v4h = v4[:, :, :].rearrange("p n (h c) -> p n h c", h=4)
                nc.vector.tensor_copy(v4h[:, :, :, DH:DH + 1],
                                      ones_b.unsqueeze(2).broadcast_to(
                                          (128, NT, 4, 1)))

                kmods = {}
                qmods = {}
                for p in (0, 1):
                    kmods[p] = kp.tile([128, N], f32r, tag=f"k{p}",
                                       name=f"km{p}")

                # ---- slice-granular producer items ----------------------
                # Each chunk is <= ~430ns of PE work so it fits the PE slack
                # inside one exp window; chunks use the m pool so they never
                # perturb the S^T psum slot rotation.
                mstate = {}
                def proj_chunk(key, w, dest_cb, srcs, half):
                    # generic 4-matmul [128,512] projection split in halves
                    if half == 0:
                        ps = mixp.tile([128, 512], fp32, tag="m",
                                       name=f"m_{key[0]}{key[1]}")[:, :]
                        mstate[key] = ps
                        ts = (0, 1)
                    else:
                        ps = mstate[key]
                        ts = (2, 3)
                    for t in ts:
                        nc.tensor.matmul(ps, w(t), srcs(t),
                                         start=(t == 0), stop=(t == DT - 1))
                    if half == 1:
                        dest_cb(ps)

                def okv_chunk(ns, half):
                    sl = slice(ns * 512, (ns + 1) * 512)

                    def dest(ps):
                        with lp(reason="f32r modulation"):
                            nc.vector.tensor_scalar_add(
                                okvT2[:, sl], ps, bfc2[:, :])
                    proj_chunk(("okv", ns), lambda t: wfc2[:, t, :], dest,
                               lambda t: yt[t][:, sl], half)

                def oq_chunk(ns, half):
                    sl = slice(ns * 512, (ns + 1) * 512)

                    def dest(ps):
                        with lp(reason="f32r modulation"):
                            nc.vector.tensor_scalar_add(
                                oqT2[:, sl], ps, bfc2[:, :])
                    proj_chunk(("oq", ns), lambda t: wfc2[:, t, :], dest,
                               lambda t: xt[t][:, sl], half)

                def k_chunk(p, ns, half):
                    km = kmods[p]
                    pf = slice(p * 128, (p + 1) * 128)
                    sl = slice(ns * 512, (ns + 1) * 512)

                    def dest(ps):
                        with lp(reason="f32r qk"):
                            nc.vector.tensor_mul(km[:, sl], ps,
                                                 okvT2[:, sl])
                    proj_chunk(("k", p, ns), lambda t: wk[:, t, pf], dest,
                               lambda t: xt[t][:, sl], half)

                def q_chunk(p, qt, half):
                    sl = slice(qt * 512, (qt + 1) * 512)
                    pf = slice(p * 128, (p + 1) * 128)
                    if half == 0:
                        qm = qmp.tile([128, 512], f32r, tag=f"q{p}",
                                      name=f"qm{p}_{qt}")
                        qmods[(p, qt)] = qm

                    def dest(ps):
                        with lp(reason="f32r qk"):
                            nc.vector.tensor_mul(qmods[(p, qt)][:, :],
                                                 ps, oqT2[:, sl])
                    proj_chunk(("q", p, qt), lambda t: wq[:, t, pf], dest,
                               lambda t: xt[t][:, sl], half)

                def tr_slice(ns):
                    # okv natural layout for 4 n-tiles via PE transpose
                    tps = mixp.tile([128, 512], fp32, tag="m",
                                    name=f"trps{ns}")[:, :]
                    for j in range(4):
                        nt = ns * 4 + j
                        nc.tensor.transpose(
                            tps[:, j * DH:(j + 1) * DH],
                            okvT2[0:DH, nt * 128:(nt + 1) * 128],
                            ident[0:DH, 0:DH])
                    with lp(reason="bf16 v-modulation"):
                        nc.vector.tensor_copy(
                            okvn[:, ns * 4:(ns + 1) * 4, :],
                            tps[:, 0:4 * DH].rearrange("p (n c) -> p n c",
                                                       n=4))

                def v4_chunk(ns, j):
                    # one n-tile (4 heads x 64) into half of a psum region;
                    # odd j modulates the completed pair
                    nt = ns * 4 + j
                    if j % 2 == 0:
                        psv = mixp.tile([128, 512], fp32, tag="m",
                                        name=f"vps{ns}_{j}")[:, :]
                        mstate[("v4", ns)] = psv
                    else:
                        psv = mstate[("v4", ns)]
                    csl = slice((j % 2) * 256, (j % 2) * 256 + 256)
                    for t in range(DT):
                        nc.tensor.matmul(psv[:, csl],
                                         xt[t][:, nt * 128:(nt + 1) * 128],
                                         wv[:, t, :],
                                         start=(t == 0), stop=(t == DT - 1))
                    if j % 2 == 1:
                        n0 = nt - 1
                        okb = okvn[:, n0:n0 + 2, :].unsqueeze(2).broadcast_to(
                            (128, 2, 4, DH))
                        with lp(reason="bf16 v-modulation"):
                            nc.vector.tensor_mul(
                                v4[:, n0:n0 + 2, :].rearrange(
                                    "p n (h c) -> p n h c",
                                    h=4)[:, :, :, 0:DH],
                                psv[:, :].rearrange(
                                    "p (n h c) -> p n h c", n=2, h=4),
                                okb)

                # ---- normalization / output projection items ------------
                ots = {}

                def norm_item(p, qt, h, acc):
                    # rec broadcast lands in the free partitions 64..127 of
                    # the accumulator's own PSUM bank — no new PSUM tile, so
                    # normalization cannot cycle through the s-pool rotation.
                    def run():
                        if (p, qt) not in ots:
                            ots[(p, qt)] = otp.tile(
                                [128, 512], bf16, tag=f"ot{p}",
                                name=f"ot{p}_{qt}")
                        ot = ots[(p, qt)]
                        rec = recp.tile([1, 512], bf16, tag="rec")
                        num = nsp.tile([DH, 512], bf16, tag="num")
                        with lp(reason="bf16 softmax normalization"):
                            nc.vector.tensor_copy(num[:, :], acc[0:DH, :])
                            nc.vector.reciprocal(rec[:, :], acc[64:65, :])
                            nc.tensor.matmul(acc[DH:128, :], ones_row[:, :],
                                             rec[:, :], start=True, stop=True)
                            nc.vector.tensor_mul(
                                ot[h * DH:(h + 1) * DH, :],
                                num[:, :], acc[DH:128, :])
                    return run

                def outproj_chunk(nt, half):
                    qt = nt // 4
                    j = nt % 4
                    jsl = slice(j * 128, (j + 1) * 128)
                    if half == 0:
                        ps = mixp.tile([128, 512], fp32, tag="m",
                                       name=f"ops{nt}")[:, :]
                        mstate[("op", nt)] = ps
                        nc.tensor.matmul(ps, ots[(0, qt)][:, jsl],
                                         wo[:, 0, :], start=True, stop=False)
                    else:
                        ps = mstate[("op", nt)]
                        nsl = slice(nt * 128, (nt + 1) * 128)
                        nc.tensor.matmul(ps, ots[(1, qt)][:, jsl],
                                         wo[:, 1, :], start=False, stop=True)
                        ob = obp.tile([128, 512], fp32, tag="ob")
                        nc.vector.tensor_copy(ob[:, :], ps)
                        nc.sync.dma_start(out_d[nsl, :], ob[:, :])

                # ---- inline prefix: slices 0-2 of okv/kmod/okvn/v4 ------
                okv_chunk(0, 0); okv_chunk(0, 1)
                oq_chunk(0, 0); oq_chunk(0, 1)
                k_chunk(0, 0, 0); k_chunk(0, 0, 1)
                q_chunk(0, 0, 0); q_chunk(0, 0, 1)
                tr_slice(0)
                for j in range(4):
                    v4_chunk(0, j)
                for ns in (1, 2):
                    okv_chunk(ns, 0); okv_chunk(ns, 1)
                    k_chunk(0, ns, 0); k_chunk(0, ns, 1)
                    tr_slice(ns)
                    for j in range(4):
                        v4_chunk(ns, j)

                fill = deque()
                urgent = deque()
                fill.append(lambda: okv_chunk(3, 0))
                fill.append(lambda: okv_chunk(3, 1))
                fill.append(lambda: k_chunk(0, 3, 0))
                fill.append(lambda: k_chunk(0, 3, 1))
                fill.append(lambda: tr_slice(3))
                for j in range(4):
                    fill.append(lambda j=j: v4_chunk(3, j))
                for qt in (1, 2, 3):
                    fill.append(lambda qt=qt: oq_chunk(qt, 0))
                    fill.append(lambda qt=qt: oq_chunk(qt, 1))
                    fill.append(lambda qt=qt: q_chunk(0, qt, 0))
                    fill.append(lambda qt=qt: q_chunk(0, qt, 1))
                for ns in range(NS):
                    fill.append(lambda ns=ns: k_chunk(1, ns, 0))
                    fill.append(lambda ns=ns: k_chunk(1, ns, 1))
                for qt in range(4):
                    fill.append(lambda qt=qt: q_chunk(1, qt, 0))
                    fill.append(lambda qt=qt: q_chunk(1, qt, 1))

                # ---- attention: flat skewed pipeline --------------------
                steps = [(p, qt, kt)
                         for p in (0, 1) for qt in range(4)
                         for kt in range(NT)]
                prev = None
                accs = None
                for idx, (p, qt, kt) in enumerate(steps):
                    km = kmods[p]
                    qm = qmods[(p, qt)]
                    ksl = slice(kt * 128, (kt + 1) * 128)
                    sp = sps.tile([128, 1024], fp32, tag="s",
                                  name=f"sp{idx}")
                    nc.tensor.matmul(sp[:, 0:512], km[0:DH, ksl],
                                     qm[0:DH, :], start=True, stop=True)
                    nc.tensor.matmul(sp[:, 512:1024], km[DH:128, ksl],
                                     qm[DH:128, :], start=True, stop=True)
                    if prev is not None:
                        pp, pqt, pkt, pe, pa0, pa1 = prev
                        nc.tensor.matmul(pa0[0:65, :],
                                         v4[:, pkt, pp * 130:pp * 130 + 65],
                                         pe[:, 0:512],
                                         start=(pkt == 0), stop=(pkt == NT - 1))
                        nc.tensor.matmul(pa1[0:65, :],
                                         v4[:, pkt,
                                            pp * 130 + 65:pp * 130 + 130],
                                         pe[:, 512:1024],
                                         start=(pkt == 0), stop=(pkt == NT - 1))
                        if pkt == NT - 1:
                            urgent.append(norm_item(pp, pqt, 0, pa0))
                            urgent.append(norm_item(pp, pqt, 1, pa1))
                            # output projection is deferred to the tail:
                            # in-stream PSUM-slot chunks each cost ~1us of
                            # stream stalls on HW (in-order PE + DVE-gated
                            # slot WAR), far more than their PE time.
                    if kt == 0:
                        accs = (accp.tile([128, 512], fp32, tag="acc",
                                          name=f"acc0_{p}_{qt}"),
                                accp.tile([128, 512], fp32, tag="acc",
                                          name=f"acc1_{p}_{qt}"))
                    e = ep.tile([128, 1024], bf16, tag="e")
                    nc.scalar.activation(e[:, :], sp[:, :], Exp,
                                         scale=float(SCALE))
                    prev = (p, qt, kt, e, accs[0], accs[1])
                    if urgent:
                        urgent.popleft()()
                    elif fill:
                        fill.popleft()()

                # tail: final attn@V + remaining fillers
                pp, pqt, pkt, pe, pa0, pa1 = prev
                nc.tensor.matmul(pa0[0:65, :],
                                 v4[:, pkt, pp * 130:pp * 130 + 65],
                                 pe[:, 0:512], start=False, stop=True)
                nc.tensor.matmul(pa1[0:65, :],
                                 v4[:, pkt, pp * 130 + 65:pp * 130 + 130],
                                 pe[:, 512:1024], start=False, stop=True)
                urgent.append(norm_item(pp, pqt, 0, pa0))
                urgent.append(norm_item(pp, pqt, 1, pa1))
                while urgent:
                    urgent.popleft()()
                while fill:
                    fill.popleft()()
                for nt in range(NT):
                    outproj_chunk(nt, 0)
                    outproj_chunk(nt, 1)

            if loop_n > 1:
                with tc.For_i(0, loop_n, 1) as _i:
                    body(_i)
            else:
                body()

    nc.compile()
    return nc


# --------------------------------------------------------------------------
# PJRT SPMD runner (axon path) — keeps the jitted callable for reuse
# --------------------------------------------------------------------------

class _SpmdRunner:
    def __init__(self, nc, n_cores):
        import jax
        from jax.sharding import Mesh, PartitionSpec, NamedSharding
        from jax.experimental.shard_map import shard_map
        import concourse.mybir as mybir
        from concourse import bass2jax
        from concourse.bass2jax import _bass_exec_p, install_neuronx_cc_hook

        install_neuronx_cc_hook()
        self.jax = jax
        self.nc = nc
        self.n_cores = n_cores
        pname = nc.partition_id_tensor.name if nc.partition_id_tensor else None
        in_names, out_names, out_avals, zero_shapes = [], [], [], []
        for alloc in nc.m.functions[0].allocations:
            if not isinstance(alloc, mybir.MemoryLocationSet):
                continue
            name = alloc.memorylocations[0].name
            if alloc.kind == "ExternalInput":
                if name != pname:
                    in_names.append(name)
            elif alloc.kind == "ExternalOutput":
                out_names.append(name)
                shape = tuple(alloc.tensor_shape)
                dtype = mybir.dt.np(alloc.dtype)
                out_avals.append(jax.core.ShapedArray(shape, dtype))
                zero_shapes.append((shape, dtype))
        self.n_params = len(in_names)
        self.in_names = list(in_names)
        self.out_names = out_names
        self.out_avals = out_avals
        all_names = in_names + out_names
        if pname is not None:
            all_names.append(pname)

        def _body(*args):
            operands = list(args)
            if pname is not None:
                operands.append(bass2jax.partition_id_tensor())
            return tuple(_bass_exec_p.bind(
                *operands, out_avals=tuple(out_avals),
                in_names=tuple(all_names), out_names=tuple(out_names),
                lowering_input_output_aliases=(),
                sim_require_finite=True, sim_require_nnan=True, nc=nc))

        devices = jax.devices()[:n_cores]
        self.mesh = Mesh(np.asarray(devices), ("core",))
        n_outs = len(out_avals)
        in_specs = (PartitionSpec("core"),) * (self.n_params + n_outs)
        out_specs = (PartitionSpec("core"),) * n_outs
        donate = tuple(range(self.n_params, self.n_params + n_outs))
        self.sharding = NamedSharding(self.mesh, PartitionSpec("core"))
        self.sharded = jax.jit(
            shard_map(_body, mesh=self.mesh, in_specs=in_specs,
                      out_specs=out_specs, check_rep=False),
            donate_argnums=donate, keep_unused=True)
        zs = [(n_cores * s[0], *s[1:]) for s, _ in zero_shapes]
        zd = [d for _, d in zero_shapes]
        self._mkzeros = jax.jit(
            lambda: tuple(jax.numpy.zeros(s, d) for s, d in zip(zs, zd)),
            out_shardings=tuple(self.sharding for _ in zs))

    def put_inputs(self, in_maps):
        concat = [np.concatenate(
            [np.ascontiguousarray(in_maps[c][n]) for c in range(self.n_cores)],
            axis=0) for n in self.in_names]
        return [self.jax.device_put(a, self.sharding) for a in concat]

    def run(self, in_dev):
        outs = self.sharded(*in_dev, *self._mkzeros())
        self.jax.block_until_ready(outs)
        return outs

    def results(self, outs):
        res = []
        for c in range(self.n_cores):
            d = {}
            for i, name in enumerate(self.out_names):
                full = np.asarray(outs[i])
                d[name] = full.reshape(self.n_cores,
                                       *self.out_avals[i].shape)[c]
            res.append(d)
        return res


def _get_runner(loop_n: int = 1):
    if loop_n not in _RUNNER_CACHE:
        nc = _build_nc(loop_n)
        _RUNNER_CACHE[loop_n] = _SpmdRunner(nc, N_CORES)
    return _RUNNER_CACHE[loop_n]


# --------------------------------------------------------------------------
# host-side shard / gather
# --------------------------------------------------------------------------

def _shard_inputs(x, y, W_qkv, W_fc, b_fc, W_out):
    import ml_dtypes
    bf = ml_dtypes.bfloat16
    in_maps = []
    x = np.asarray(x)
    y = np.asarray(y)
    W_qkv = np.asarray(W_qkv)
    W_fc = np.ascontiguousarray(np.asarray(W_fc, dtype=np.float32))
    b_fc = np.asarray(b_fc, dtype=np.float32).reshape(DH, 1)
    W_out = np.asarray(W_out)
    xTb = [np.ascontiguousarray(x[b].T) for b in range(B)]
    yTb = [np.ascontiguousarray(y[b].T) for b in range(B)]
    for c in range(N_CORES):
        b, g = c // 2, c % 2
        hs = slice(g * 256, (g + 1) * 256)
        in_maps.append({
            "xT": xTb[b],
            "yT": yTb[b],
            "wq": np.ascontiguousarray(W_qkv[:, hs]),
            "wk": np.ascontiguousarray(W_qkv[:, 512:][:, hs]),
            "wv": np.ascontiguousarray(W_qkv[:, 1024:][:, hs]),
            "wfc": W_fc,
            "bfc": b_fc,
            "wo": np.ascontiguousarray(W_out[hs, :]).astype(bf),
        })
    return in_maps


def kernel(x, y, W_qkv, W_fc, b_fc, W_out, b_out):
    runner = _get_runner(1)
    in_maps = _shard_inputs(x, y, W_qkv, W_fc, b_fc, W_out)
    in_dev = runner.put_inputs(in_maps)
    res = runner.results(runner.run(in_dev))
    b_out = np.asarray(b_out, dtype=np.float32)
    out = np.empty((B, N, DIM), dtype=np.float32)
    for b in range(B):
        out[b] = res[2 * b]["out"] + res[2 * b + 1]["out"] + b_out
    return out
